# revision 24
# baseline (speedup 1.0000x reference)
"""DGCNN forward on 8 Trainium2 cores (self-contained).

500 graphs (200 nodes, block-diag edges) padded to 512, 64 graphs/core in 16
groups of 4.  All four GCN layers run on device at fp32-equivalent precision
via bf16 hi/lo pair arithmetic:

  transform  t = h @ W   2 matmuls: K-stacked lhsT [h_hi; h_lo] with rhs
                         [W_hi; W_lo] then swapped -> all 4 cross terms
                         accumulate in psum (exact to ~2^-17).
  propagate  p = A_n @ t feat-major: lhsT = [t_hi | t_lo] (64 wide, strided
                         AP), stream A^T_hi then A^T_lo per 128/72 src chunk;
                         psum rows 0-31/32-63 hold hi/lo partials, summed by
                         shift-copy + DVE add.  h = tanh(p) on scalar.

Two graphs per psum/staging tile at partition bases 0/64 keep all engine op
partition bases equal (hw requirement).  Groups are processed layer-major in
two sets of 8 so independent per-group chains interleave in the FIFO engine
queues.  Host does the cheap tail (top-30 sort, convs, MLP) in fp32 numpy.
"""
import os
import numpy as np
import ml_dtypes

N_GRAPHS, N_PER, K_TOP, F_IN, H = 500, 200, 30, 128, 32
G_PAD = 512
G_CORE = 64
NGRP = 16
NSET = 8            # groups per resident set
BF16 = ml_dtypes.bfloat16
C0, C1 = 128, 72


def _build_adj(edge_index):
    n = N_GRAPHS * N_PER
    src = np.concatenate([edge_index[0].astype(np.int64), np.arange(n, dtype=np.int64)])
    dst = np.concatenate([edge_index[1].astype(np.int64), np.arange(n, dtype=np.int64)])
    deg = np.bincount(dst, minlength=n).astype(np.float32)
    inv = (1.0 / np.sqrt(np.maximum(deg, 1e-12))).astype(np.float32)
    w = (inv[src] * inv[dst]).astype(np.float32)
    A = np.zeros((N_GRAPHS, N_PER, N_PER), np.float32)
    np.add.at(A, (dst // N_PER, dst % N_PER, src % N_PER), w)
    return A


def _host_tail(hcat, inputs):
    G = hcat.shape[0]
    order = np.argsort(-hcat[:, :, -1], axis=1, kind="stable")[:, :K_TOP]
    topk = np.take_along_axis(hcat, order[:, :, None], axis=1)
    C1w = np.asarray(inputs["cw1"], np.float32)[:, 0, :].T
    c1 = np.maximum(np.einsum("gkc,co->gko", topk, C1w) + np.asarray(inputs["cb1"], np.float32), 0)
    p1 = np.maximum(c1[:, 0::2, :], c1[:, 1::2, :])
    cw2 = np.asarray(inputs["cw2"], np.float32)
    c2 = np.zeros((G, 11, 32), np.float32)
    for k in range(5):
        c2 += np.einsum("gti,io->gto", p1[:, k:k + 11, :], cw2[:, :, k].T)
    c2 = np.maximum(c2 + np.asarray(inputs["cb2"], np.float32), 0)
    flat = c2.transpose(0, 2, 1).reshape(G, -1)
    z = np.maximum(flat @ np.asarray(inputs["lw1"], np.float32) + np.asarray(inputs["lb1"], np.float32), 0)
    o = z @ np.asarray(inputs["lw2"], np.float32) + np.asarray(inputs["lb2"], np.float32)
    return (1.0 / (1.0 + np.exp(-o))).astype(np.float32)


def _split(a):
    hi = a.astype(BF16)
    lo = (a - hi.astype(np.float32)).astype(BF16)
    return hi, lo


def _device_gcn(ins):
    import concourse.bacc as bacc
    import concourse.mybir as mybir
    import concourse.tile as tile
    from concourse import bass_utils

    dt = mybir.dt
    ACT = mybir.ActivationFunctionType
    OP = mybir.AluOpType
    nc = bacc.Bacc("TRN2", target_bir_lowering=False, debug=False, num_devices=8)

    d = {}
    for name, shape, ddt in [
        ("aX", (NGRP, 128, 1600), dt.bfloat16),   # [ahi0 | alo0]
        ("aY", (NGRP, 72, 1600), dt.bfloat16),    # [ahi1 | alo1]
        ("tl0", (NGRP, 128, 4, 2, 2, 32), dt.bfloat16),  # host-computed x@W1 pair
        ("w2r", (64, 32), dt.bfloat16), ("w2s", (64, 32), dt.bfloat16),
        ("w3r", (64, 32), dt.bfloat16), ("w3s", (64, 32), dt.bfloat16),
        ("w4r", (64, 1), dt.bfloat16), ("w4s", (64, 1), dt.bfloat16),
    ]:
        d[name] = nc.dram_tensor(name, shape, ddt, kind="ExternalInput").ap()
    # H: [grp, layer, pr, 32feat, par, 200node]
    d_H = nc.dram_tensor("H", (NGRP, 3, 64, 2, 200), dt.float32, kind="ExternalOutput").ap()
    # H4: [grp, pr, hi/lo, par, 200] pre-tanh
    d_H4 = nc.dram_tensor("H4", (NGRP, 34, 2, 200), dt.float32, kind="ExternalOutput").ap()

    with tile.TileContext(nc) as tc:
        with tc.tile_pool(name="wp", bufs=1) as wp, \
             tc.tile_pool(name="ain", bufs=2) as ain, \
             tc.tile_pool(name="ainx", bufs=1) as ainx, \
             tc.tile_pool(name="sb", bufs=3) as sb, \
             tc.tile_pool(name="hhp", bufs=1) as hhp, \
             tc.tile_pool(name="pst", bufs=2, space="PSUM") as pst, \
             tc.tile_pool(name="psp", bufs=2, space="PSUM") as psp:
            W = {}
            for name in ["w2r", "w2s", "w3r", "w3s"]:
                W[name] = wp.tile([64, 32], dt.bfloat16, name=name)
                nc.sync.dma_start(out=W[name][:], in_=d[name])
            for name in ["w4r", "w4s"]:
                W[name] = wp.tile([64, 1], dt.bfloat16, name=name)
                nc.sync.dma_start(out=W[name][:], in_=d[name])
            WR = [None, (W["w2r"], W["w2s"]), (W["w3r"], W["w3s"]), (W["w4r"], W["w4s"])]

            for st in range(NGRP // NSET):
                AX, AY, XT = {}, {}, {}
                for sl in range(NSET):
                    grp = st * NSET + sl
                    AX[sl] = ain.tile([128, 1600], dt.bfloat16, tag=f"aX{sl}", name=f"aX{sl}")
                    nc.sync.dma_start(out=AX[sl][:], in_=d["aX"][grp])
                    AY[sl] = ain.tile([72, 1600], dt.bfloat16, tag=f"aY{sl}", name=f"aY{sl}")
                    nc.sync.dma_start(out=AY[sl][:], in_=d["aY"][grp])
                    XT[sl] = ainx.tile([128, 4, 2, 2, 32], dt.bfloat16, tag=f"tl0{sl}", name=f"tl0{sl}")
                    nc.sync.dma_start(out=XT[sl][:], in_=d["tl0"][grp])
                hh_prev = {}
                for l in range(4):
                    tw = 1 if l == 3 else 32
                    HTd = {}

                    def emit_T(sl, pr):
                        tls = []
                        for par in range(2):
                            if l == 0:
                                tls.append(XT[sl][:, 2 * pr + par])
                                continue
                            t2 = pst.tile([128, 2, tw], dt.float32, tag=f"t2{par}", name=f"t2{par}")
                            hh = hh_prev[(sl, pr)]
                            wr = WR[l]
                            for c, cn in enumerate((C0, C1)):
                                out = t2[0:cn, c, :]
                                hsl = hh[0:64, par, c * 128:c * 128 + cn]
                                nc.tensor.matmul(out, lhsT=hsl, rhs=wr[0][:, 0:tw],
                                                 start=True, stop=False)
                                nc.tensor.matmul(out, lhsT=hsl, rhs=wr[1][:, 0:tw],
                                                 start=False, stop=True)
                            tl = sb.tile([128, 2, 2, tw], dt.bfloat16, tag=f"tl{par}", name=f"tl{par}")
                            nc.scalar.activation(tl[:, :, 0, :], t2[:, :, :], ACT.Copy)
                            nc.vector.tensor_tensor(tl[:, :, 1, :], t2[:, :, :], tl[:, :, 0, :], OP.subtract)
                            tls.append(tl)
                        return tls

                    def emit_P(sl, pr, tls):
                        grp = st * NSET + sl
                        if pr == 0:
                            tag = "HT" if l < 3 else "ps4"
                            HTd[sl] = sb.tile([64, 2, 200], dt.float32, tag=tag, name=tag)
                        HT = HTd[sl]
                        p2 = psp.tile([64, 2, 200], dt.float32, tag=f"p2{pr}", name=f"p2{pr}")
                        for par in range(2):
                            g = 2 * pr + par
                            gc = slice(200 * g, 200 * g + 200)
                            gc2 = slice(800 + 200 * g, 800 + 200 * g + 200)
                            tl = tls[par]
                            pout = p2[0:2 * tw, par, :]
                            lh0 = tl[:, 0]
                            lh1 = tl[0:72, 1]
                            nc.tensor.matmul(pout, lhsT=lh0, rhs=AX[sl][:, gc],
                                             start=True, stop=False)
                            nc.tensor.matmul(pout, lhsT=lh0, rhs=AX[sl][:, gc2],
                                             start=False, stop=False)
                            nc.tensor.matmul(pout, lhsT=lh1, rhs=AY[sl][0:72, gc],
                                             start=False, stop=False)
                            nc.tensor.matmul(pout, lhsT=lh1, rhs=AY[sl][0:72, gc2],
                                             start=False, stop=True)
                        if l == 3:
                            nc.scalar.activation(HT[32 * pr:32 * pr + 2, :, :], p2[0:2, :, :], ACT.Copy)
                            if pr == 1:
                                nc.gpsimd.dma_start(out=d_H4[grp], in_=HT[0:34])
                            return
                        tmp = sb.tile([32, 2, 200], dt.float32, tag=f"tmp{pr}", name=f"tmp{pr}")
                        nc.vector.tensor_copy(tmp[:, :, :], p2[32:64, :, :])
                        P = sb.tile([32, 2, 200], dt.float32, tag=f"P{pr}", name=f"P{pr}")
                        nc.vector.tensor_tensor(P[:, :, :], p2[0:32, :, :], tmp[:, :, :], OP.add)
                        b = 32 * pr
                        nc.scalar.activation(HT[b:b + 32, :, :], P[:, :, :], ACT.Tanh)
                        hh = hhp.tile([64, 2, 200], dt.bfloat16, tag=f"hh{sl}{pr}", name=f"hh{sl}{pr}")
                        if pr == 0:
                            nc.scalar.activation(hh[0:32, :, :], HT[0:32, :, :], ACT.Copy)
                            nc.vector.tensor_tensor(hh[32:64, :, :], HT[0:32, :, :], hh[0:32, :, :], OP.subtract)
                        else:
                            nc.gpsimd.tensor_copy(hh[32:64, :, :], HT[32:64, :, :])
                            nc.gpsimd.tensor_tensor(hh[0:32, :, :], HT[32:64, :, :], hh[32:64, :, :], OP.subtract)
                            nc.gpsimd.dma_start(out=d_H[grp, l], in_=HT[:])
                        hh_prev[(sl, pr)] = hh

                    pend = None
                    for sl in range(NSET):
                        for pr in range(2):
                            tls = emit_T(sl, pr)
                            if pend is not None:
                                emit_P(*pend)
                            pend = (sl, pr, tls)
                    emit_P(*pend)

    nc.compile()

    trace = bool(int(os.environ.get("BASS_KERNEL_TRACE", "0")))
    res = bass_utils.run_bass_kernel_spmd(nc, ins, core_ids=list(range(8)), trace=trace)
    if trace and res.exec_time_ns is not None:
        print(f"HW exec time: {res.exec_time_ns} ns")
    return res.results


def kernel(**inputs):
    x = np.asarray(inputs["x"], np.float32)
    ei = np.asarray(inputs["edge_index"])
    A = _build_adj(ei)
    Ws = [np.asarray(inputs[f"W{i}"], np.float32) for i in (1, 2, 3, 4)]
    bs = [np.asarray(inputs[f"b{i}"], np.float32) for i in (1, 2, 3, 4)]
    xg = x.reshape(N_GRAPHS, N_PER, F_IN)

    use_device = all(np.all(b == 0) for b in bs)
    hcat = None
    if use_device:
        try:
            At = np.zeros((G_PAD, N_PER, N_PER), np.float32)
            At[:N_GRAPHS] = A.transpose(0, 2, 1)
            Ahi, Alo = _split(At)
            t1 = np.zeros((G_PAD, N_PER, 32), np.float32)
            t1[:N_GRAPHS] = np.einsum("gnf,fo->gno", xg.astype(np.float64), Ws[0].astype(np.float64), optimize=True).astype(np.float32)
            t1hi, t1lo = _split(t1)
            TL = np.zeros((G_PAD, 128, 2, 2, 32), BF16)
            TL[:, :, 0, 0] = t1hi[:, 0:128]
            TL[:, :, 0, 1] = t1lo[:, 0:128]
            TL[:, 0:72, 1, 0] = t1hi[:, 128:200]
            TL[:, 0:72, 1, 1] = t1lo[:, 128:200]

            def core_view(arr, rows):
                # arr [512, rows, 200] bf16 -> [8, NGRP, rows, 800]
                return (arr.reshape(8, NGRP, 4, rows, 200)
                           .transpose(0, 1, 3, 2, 4).reshape(8, NGRP, rows, 800).copy())

            ahi0 = core_view(np.ascontiguousarray(Ahi[:, 0:128]), 128)
            alo0 = core_view(np.ascontiguousarray(Alo[:, 0:128]), 128)
            ahi1 = core_view(np.ascontiguousarray(Ahi[:, 128:200]), 72)
            alo1 = core_view(np.ascontiguousarray(Alo[:, 128:200]), 72)
            aX = np.concatenate([ahi0, alo0], axis=3)      # [8, NGRP, 128, 1600]
            aY = np.concatenate([ahi1, alo1], axis=3)      # [8, NGRP, 72, 1600]
            tl0 = (TL.reshape(8, NGRP, 4, 128, 2, 2, 32)
                     .transpose(0, 1, 3, 2, 4, 5, 6).copy())  # [8,16,128,4,2,2,32]

            def wpair(Wm, rep):
                hi, lo = _split(Wm)
                pair = np.concatenate([hi.astype(np.float32), lo.astype(np.float32)], axis=0)
                swap = np.concatenate([lo.astype(np.float32), hi.astype(np.float32)], axis=0)
                if rep:
                    pair = np.concatenate([pair, pair], axis=0)
                    swap = np.concatenate([swap, swap], axis=0)
                return pair.astype(BF16), swap.astype(BF16)

            w2r, w2s = wpair(Ws[1], False)
            w3r, w3s = wpair(Ws[2], False)
            w4r, w4s = wpair(Ws[3], False)

            ins = [{"aX": aX[c], "aY": aY[c], "tl0": tl0[c],
                    "w2r": w2r, "w2s": w2s, "w3r": w3r, "w3s": w3s,
                    "w4r": w4r, "w4s": w4s} for c in range(8)]
            res = _device_gcn(ins)

            hs = []
            for l in range(3):
                v = np.stack([res[c]["H"][:, l] for c in range(8)])   # [8,16,64,2par,200]
                v = v.reshape(8, NGRP, 2, 32, 2, 200)                  # [.., pr, feat, par, node]
                v = v.transpose(0, 1, 2, 4, 5, 3)                      # [.., pr, par, node, feat]
                hs.append(v.reshape(G_PAD, N_PER, 32)[:N_GRAPHS])
            v4 = np.stack([res[c]["H4"] for c in range(8)])           # [8,16,34,2par,200]
            h4 = np.tanh(v4[:, :, [0, 32]] + v4[:, :, [1, 33]])       # [8,16,2pr,2par,200]
            h4 = h4.reshape(G_PAD, N_PER, 1)[:N_GRAPHS]
            hcat = np.concatenate(hs + [h4], axis=-1)
        except Exception as e:
            print("device path failed, falling back to host:", repr(e))
            hcat = None
    if hcat is None:
        h = xg
        hs = []
        for l in range(4):
            h = np.tanh(np.einsum("gds,gsf->gdf", A, h) @ Ws[l] + bs[l])
            hs.append(h)
        hcat = np.concatenate(hs, axis=-1)
    return _host_tail(hcat, inputs)


# revision 25
# speedup vs baseline: 1.1698x; 1.1698x over previous
"""DGCNN forward on 8 Trainium2 cores (self-contained).

500 graphs (200 nodes, block-diag edges) padded to 512, 64 graphs/core in 16
groups of 4.  All four GCN layers run on device at fp32-equivalent precision
via bf16 hi/lo pair arithmetic:

  transform  t = h @ W   2 matmuls: K-stacked lhsT [h_hi; h_lo] with rhs
                         [W_hi; W_lo] then swapped -> all 4 cross terms
                         accumulate in psum (exact to ~2^-17).
  propagate  p = A_n @ t feat-major: lhsT = [t_hi | t_lo] (64 wide, strided
                         AP), stream A^T_hi then A^T_lo per 128/72 src chunk;
                         psum rows 0-31/32-63 hold hi/lo partials, summed by
                         shift-copy + DVE add.  h = tanh(p) on scalar.

Two graphs per psum/staging tile at partition bases 0/64 keep all engine op
partition bases equal (hw requirement).  Groups are processed layer-major in
two sets of 8 so independent per-group chains interleave in the FIFO engine
queues.  Host does the cheap tail (top-30 sort, convs, MLP) in fp32 numpy.
"""
import os
import numpy as np
import ml_dtypes

N_GRAPHS, N_PER, K_TOP, F_IN, H = 500, 200, 30, 128, 32
G_PAD = 512
G_CORE = 64
NGRP = 16
NSET = 8            # groups per resident set
BF16 = ml_dtypes.bfloat16
C0, C1 = 128, 72


def _build_adj(edge_index):
    n = N_GRAPHS * N_PER
    src = np.concatenate([edge_index[0].astype(np.int64), np.arange(n, dtype=np.int64)])
    dst = np.concatenate([edge_index[1].astype(np.int64), np.arange(n, dtype=np.int64)])
    deg = np.bincount(dst, minlength=n).astype(np.float32)
    inv = (1.0 / np.sqrt(np.maximum(deg, 1e-12))).astype(np.float32)
    w = (inv[src] * inv[dst]).astype(np.float32)
    A = np.zeros((N_GRAPHS, N_PER, N_PER), np.float32)
    np.add.at(A, (dst // N_PER, dst % N_PER, src % N_PER), w)
    return A


def _host_tail(hcat, inputs):
    G = hcat.shape[0]
    order = np.argsort(-hcat[:, :, -1], axis=1, kind="stable")[:, :K_TOP]
    topk = np.take_along_axis(hcat, order[:, :, None], axis=1)
    C1w = np.asarray(inputs["cw1"], np.float32)[:, 0, :].T
    c1 = np.maximum(np.einsum("gkc,co->gko", topk, C1w) + np.asarray(inputs["cb1"], np.float32), 0)
    p1 = np.maximum(c1[:, 0::2, :], c1[:, 1::2, :])
    cw2 = np.asarray(inputs["cw2"], np.float32)
    c2 = np.zeros((G, 11, 32), np.float32)
    for k in range(5):
        c2 += np.einsum("gti,io->gto", p1[:, k:k + 11, :], cw2[:, :, k].T)
    c2 = np.maximum(c2 + np.asarray(inputs["cb2"], np.float32), 0)
    flat = c2.transpose(0, 2, 1).reshape(G, -1)
    z = np.maximum(flat @ np.asarray(inputs["lw1"], np.float32) + np.asarray(inputs["lb1"], np.float32), 0)
    o = z @ np.asarray(inputs["lw2"], np.float32) + np.asarray(inputs["lb2"], np.float32)
    return (1.0 / (1.0 + np.exp(-o))).astype(np.float32)


def _split(a):
    hi = a.astype(BF16)
    lo = (a - hi.astype(np.float32)).astype(BF16)
    return hi, lo


def _device_gcn(ins):
    import concourse.bacc as bacc
    import concourse.mybir as mybir
    import concourse.tile as tile
    from concourse import bass_utils

    dt = mybir.dt
    ACT = mybir.ActivationFunctionType
    OP = mybir.AluOpType
    nc = bacc.Bacc("TRN2", target_bir_lowering=False, debug=False, num_devices=8)

    d = {}
    for name, shape, ddt in [
        ("aX", (NGRP, 128, 1600), dt.bfloat16),   # [ahi0 | alo0]
        ("aY", (NGRP, 72, 1600), dt.bfloat16),    # [ahi1 | alo1]
        ("tl0", (NGRP, 128, 4, 2, 2, 32), dt.bfloat16),  # host-computed x@W1 pair
        ("w2r", (64, 32), dt.bfloat16), ("w2s", (64, 32), dt.bfloat16),
        ("w3r", (64, 32), dt.bfloat16), ("w3s", (64, 32), dt.bfloat16),
        ("w4r", (64, 1), dt.bfloat16), ("w4s", (64, 1), dt.bfloat16),
    ]:
        d[name] = nc.dram_tensor(name, shape, ddt, kind="ExternalInput").ap()
    # H: [grp, layer, pr, 32feat, par, 200node]
    d_H = nc.dram_tensor("H", (NGRP, 3, 64, 2, 200), dt.float32, kind="ExternalOutput").ap()
    # H4: [grp, pr, hi/lo, par, 200] pre-tanh
    d_H4 = nc.dram_tensor("H4", (NGRP, 34, 2, 200), dt.float32, kind="ExternalOutput").ap()

    with tile.TileContext(nc) as tc:
        with tc.tile_pool(name="wp", bufs=1) as wp, \
             tc.tile_pool(name="ain", bufs=2) as ain, \
             tc.tile_pool(name="ainx", bufs=1) as ainx, \
             tc.tile_pool(name="sb", bufs=3) as sb, \
             tc.tile_pool(name="hhp", bufs=1) as hhp, \
             tc.tile_pool(name="pst", bufs=2, space="PSUM") as pst, \
             tc.tile_pool(name="psp", bufs=2, space="PSUM") as psp:
            W = {}
            for name in ["w2r", "w2s", "w3r", "w3s"]:
                W[name] = wp.tile([64, 32], dt.bfloat16, name=name)
                nc.sync.dma_start(out=W[name][:], in_=d[name])
            for name in ["w4r", "w4s"]:
                W[name] = wp.tile([64, 1], dt.bfloat16, name=name)
                nc.sync.dma_start(out=W[name][:], in_=d[name])
            WR = [None, (W["w2r"], W["w2s"]), (W["w3r"], W["w3s"]), (W["w4r"], W["w4s"])]

            for st in range(NGRP // NSET):
                AX, AY, XT = {}, {}, {}
                for sl in range(NSET):
                    grp = st * NSET + sl
                    AX[sl] = ain.tile([128, 1600], dt.bfloat16, tag=f"aX{sl}", name=f"aX{sl}")
                    nc.sync.dma_start(out=AX[sl][:], in_=d["aX"][grp])
                    AY[sl] = ain.tile([72, 1600], dt.bfloat16, tag=f"aY{sl}", name=f"aY{sl}")
                    nc.sync.dma_start(out=AY[sl][:], in_=d["aY"][grp])
                    XT[sl] = ainx.tile([128, 4, 2, 2, 32], dt.bfloat16, tag=f"tl0{sl}", name=f"tl0{sl}")
                    nc.sync.dma_start(out=XT[sl][:], in_=d["tl0"][grp])
                hh_prev = {}
                for l in range(4):
                    tw = 1 if l == 3 else 32
                    for sl in range(NSET):
                        grp = st * NSET + sl
                        HT = None
                        if l < 3:
                            HT = sb.tile([64, 2, 200], dt.float32, tag="HT", name="HT")
                        else:
                            HT = sb.tile([64, 2, 200], dt.float32, tag="ps4", name="ps4")
                        for pr in range(2):
                            # p2 [2*tw rows, par, 200]: both pair graphs at base 0
                            p2 = psp.tile([64, 2, 200], dt.float32, tag=f"p2{pr}", name=f"p2{pr}")
                            tls = []
                            for par in range(2):
                                if l == 0:
                                    tls.append(XT[sl][:, 2 * pr + par])
                                    continue
                                t2 = pst.tile([128, 2, tw], dt.float32, tag=f"t2{par}", name=f"t2{par}")
                                hh = hh_prev[(sl, pr)]
                                wr = WR[l]
                                for c, cn in enumerate((C0, C1)):
                                    out = t2[0:cn, c, :]
                                    hsl = hh[0:64, par, c * 128:c * 128 + cn]
                                    nc.tensor.matmul(out, lhsT=hsl, rhs=wr[0][:, 0:tw],
                                                     start=True, stop=False)
                                    nc.tensor.matmul(out, lhsT=hsl, rhs=wr[1][:, 0:tw],
                                                     start=False, stop=True)
                                # split t: tl [128, c, hi/lo, tw]
                                tl = sb.tile([128, 2, 2, tw], dt.bfloat16, tag=f"tl{par}", name=f"tl{par}")
                                nc.scalar.activation(tl[:, :, 0, :], t2[:, :, :], ACT.Copy)
                                nc.vector.tensor_tensor(tl[:, :, 1, :], t2[:, :, :], tl[:, :, 0, :], OP.subtract)
                                tls.append(tl)
                            for par in range(2):
                                g = 2 * pr + par
                                gc = slice(200 * g, 200 * g + 200)
                                gc2 = slice(800 + 200 * g, 800 + 200 * g + 200)
                                tl = tls[par]
                                # propagate into col block `par`
                                pout = p2[0:2 * tw, par, :]
                                lh0 = tl[:, 0]                  # [c0hi | c0lo] contiguous
                                lh1 = tl[0:72, 1]               # [c1hi | c1lo] contiguous
                                nc.tensor.matmul(pout, lhsT=lh0, rhs=AX[sl][:, gc],
                                                 start=True, stop=False)
                                nc.tensor.matmul(pout, lhsT=lh0, rhs=AX[sl][:, gc2],
                                                 start=False, stop=False)
                                nc.tensor.matmul(pout, lhsT=lh1, rhs=AY[sl][0:72, gc],
                                                 start=False, stop=False)
                                nc.tensor.matmul(pout, lhsT=lh1, rhs=AY[sl][0:72, gc2],
                                                 start=False, stop=True)
                            if l == 3:
                                nc.scalar.activation(HT[32 * pr:32 * pr + 2, :, :], p2[0:2, :, :], ACT.Copy)
                                if pr == 1:
                                    nc.gpsimd.dma_start(out=d_H4[grp], in_=HT[0:34])
                                continue
                            # pair-sum + tanh (single merged ops over both par blocks)
                            tmp = sb.tile([32, 2, 200], dt.float32, tag=f"tmp{pr}", name=f"tmp{pr}")
                            nc.vector.tensor_copy(tmp[:, :, :], p2[32:64, :, :])
                            P = sb.tile([32, 2, 200], dt.float32, tag=f"P{pr}", name=f"P{pr}")
                            nc.vector.tensor_tensor(P[:, :, :], p2[0:32, :, :], tmp[:, :, :], OP.add)
                            b = 32 * pr
                            nc.scalar.activation(HT[b:b + 32, :, :], P[:, :, :], ACT.Tanh)
                            # split h: hh rows [hi; lo] (pr0) / [lo; hi] (pr1) - order
                            # is irrelevant since both W-pair rhs tiles accumulate
                            hh = hhp.tile([64, 2, 200], dt.bfloat16, tag=f"hh{sl}{pr}", name=f"hh{sl}{pr}")
                            if pr == 0:
                                nc.scalar.activation(hh[0:32, :, :], HT[0:32, :, :], ACT.Copy)
                                nc.vector.tensor_tensor(hh[32:64, :, :], HT[0:32, :, :], hh[0:32, :, :], OP.subtract)
                            else:
                                nc.gpsimd.tensor_copy(hh[32:64, :, :], HT[32:64, :, :])
                                nc.gpsimd.tensor_tensor(hh[0:32, :, :], HT[32:64, :, :], hh[32:64, :, :], OP.subtract)
                                nc.gpsimd.dma_start(out=d_H[grp, l], in_=HT[:])
                            hh_prev[(sl, pr)] = hh

    nc.compile()

    trace = bool(int(os.environ.get("BASS_KERNEL_TRACE", "0")))
    res = bass_utils.run_bass_kernel_spmd(nc, ins, core_ids=list(range(8)), trace=trace)
    if trace and res.exec_time_ns is not None:
        print(f"HW exec time: {res.exec_time_ns} ns")
    return res.results


def kernel(**inputs):
    x = np.asarray(inputs["x"], np.float32)
    ei = np.asarray(inputs["edge_index"])
    A = _build_adj(ei)
    Ws = [np.asarray(inputs[f"W{i}"], np.float32) for i in (1, 2, 3, 4)]
    bs = [np.asarray(inputs[f"b{i}"], np.float32) for i in (1, 2, 3, 4)]
    xg = x.reshape(N_GRAPHS, N_PER, F_IN)

    use_device = all(np.all(b == 0) for b in bs)
    hcat = None
    if use_device:
        try:
            At = np.zeros((G_PAD, N_PER, N_PER), np.float32)
            At[:N_GRAPHS] = A.transpose(0, 2, 1)
            Ahi, Alo = _split(At)
            t1 = np.zeros((G_PAD, N_PER, 32), np.float32)
            t1[:N_GRAPHS] = np.einsum("gnf,fo->gno", xg.astype(np.float64), Ws[0].astype(np.float64), optimize=True).astype(np.float32)
            t1hi, t1lo = _split(t1)
            TL = np.zeros((G_PAD, 128, 2, 2, 32), BF16)
            TL[:, :, 0, 0] = t1hi[:, 0:128]
            TL[:, :, 0, 1] = t1lo[:, 0:128]
            TL[:, 0:72, 1, 0] = t1hi[:, 128:200]
            TL[:, 0:72, 1, 1] = t1lo[:, 128:200]

            def core_view(arr, rows):
                # arr [512, rows, 200] bf16 -> [8, NGRP, rows, 800]
                return (arr.reshape(8, NGRP, 4, rows, 200)
                           .transpose(0, 1, 3, 2, 4).reshape(8, NGRP, rows, 800).copy())

            ahi0 = core_view(np.ascontiguousarray(Ahi[:, 0:128]), 128)
            alo0 = core_view(np.ascontiguousarray(Alo[:, 0:128]), 128)
            ahi1 = core_view(np.ascontiguousarray(Ahi[:, 128:200]), 72)
            alo1 = core_view(np.ascontiguousarray(Alo[:, 128:200]), 72)
            aX = np.concatenate([ahi0, alo0], axis=3)      # [8, NGRP, 128, 1600]
            aY = np.concatenate([ahi1, alo1], axis=3)      # [8, NGRP, 72, 1600]
            tl0 = (TL.reshape(8, NGRP, 4, 128, 2, 2, 32)
                     .transpose(0, 1, 3, 2, 4, 5, 6).copy())  # [8,16,128,4,2,2,32]

            def wpair(Wm, rep):
                hi, lo = _split(Wm)
                pair = np.concatenate([hi.astype(np.float32), lo.astype(np.float32)], axis=0)
                swap = np.concatenate([lo.astype(np.float32), hi.astype(np.float32)], axis=0)
                if rep:
                    pair = np.concatenate([pair, pair], axis=0)
                    swap = np.concatenate([swap, swap], axis=0)
                return pair.astype(BF16), swap.astype(BF16)

            w2r, w2s = wpair(Ws[1], False)
            w3r, w3s = wpair(Ws[2], False)
            w4r, w4s = wpair(Ws[3], False)

            ins = [{"aX": aX[c], "aY": aY[c], "tl0": tl0[c],
                    "w2r": w2r, "w2s": w2s, "w3r": w3r, "w3s": w3s,
                    "w4r": w4r, "w4s": w4s} for c in range(8)]
            res = _device_gcn(ins)

            hs = []
            for l in range(3):
                v = np.stack([res[c]["H"][:, l] for c in range(8)])   # [8,16,64,2par,200]
                v = v.reshape(8, NGRP, 2, 32, 2, 200)                  # [.., pr, feat, par, node]
                v = v.transpose(0, 1, 2, 4, 5, 3)                      # [.., pr, par, node, feat]
                hs.append(v.reshape(G_PAD, N_PER, 32)[:N_GRAPHS])
            v4 = np.stack([res[c]["H4"] for c in range(8)])           # [8,16,34,2par,200]
            h4 = np.tanh(v4[:, :, [0, 32]] + v4[:, :, [1, 33]])       # [8,16,2pr,2par,200]
            h4 = h4.reshape(G_PAD, N_PER, 1)[:N_GRAPHS]
            hcat = np.concatenate(hs + [h4], axis=-1)
        except Exception as e:
            print("device path failed, falling back to host:", repr(e))
            hcat = None
    if hcat is None:
        h = xg
        hs = []
        for l in range(4):
            h = np.tanh(np.einsum("gds,gsf->gdf", A, h) @ Ws[l] + bs[l])
            hs.append(h)
        hcat = np.concatenate(hs, axis=-1)
    return _host_tail(hcat, inputs)
